# revision 20
# baseline (speedup 1.0000x reference)
"""Exact top-k (k=32) attention on 8 Trainium2 NeuronCores.

Head-parallel sharding: core c computes (batch 0, head c) and (batch 1,
head c).  Per-batch key-length truncation: only SC = ceil(kl/128) chunks
of 128 keys are ever touched (the rest can never enter the top-32), so
each core's two head-slots run with different (smaller) S.

Per head, per core:
  Phase 1 (selection): forward scores F[q, s] via a 2-pass bf16-split
    matmul (hi*hi + partial lo*lo in pass A; hi*lo + lo*hi in pass BC;
    ~1e-5 accurate).  Hierarchical exact top-32: per-128-chunk top-8 via
    one DVE max8 each (level 1), then top-32 of the <=128 candidates via
    4x max8 + 3x match_replace (level 2).  A row can only be mis-extracted
    if one chunk holds >= 9 of its top-32; those rows are detected via
    flag = max_c(chunk c's 8th-largest) > t and recomputed exactly on the
    host (~1% of rows).  Cut value t_minus = t - |t|*2^-23 - 1e-37,
    strictly inside (s_33, s_32]; bf16 triple-split of -t_minus is staged
    into rows 65..67 of the qa operand.
  Phase 2 (apply): transposed scores minus t_minus computed directly by
    the same augmented matmul pair (extra contraction rows carry the mask
    and -t split), giving d'[s, q] = F^T - t_minus in PSUM.  Then
      g = Exp(temp*d')            (ScalarE, bf16)
      W = (d' > 0) * g            (GpSimd scalar_tensor_tensor)
    and AV = V_aug^T W in one matmul per chunk (V_aug has a ones column
    carrying Z = sum of selected weights); output = AV / Z.
  Emission order pipelines phase 1 of unit k+1 ahead of phase 2 of unit
  k so DVE extraction overlaps PE/Scalar/Pool apply work.
"""

import numpy as np
import ml_dtypes

N, L, S, H, E, D = 2, 2048, 2048, 8, 64, 64
TOPK = 32
TEMP = 1.0 / np.sqrt(E)
HEADS_PER_CORE = 2
N_CORES = 8
LT = 16          # L tiles of 128
QB = 4           # q groups of 512
NEG = -1e30
NLO = 60         # e-rows of the lo*lo partial correction in pass A
USE_POOL_SELECT = True

_bf16 = ml_dtypes.bfloat16


def _build_bass(sc):
    """sc: tuple (SC0, SC1) chunk counts (128 keys each) per head-slot."""
    import concourse.mybir as mybir
    from concourse import bacc
    from concourse.tile import TileContext
    from concourse.masks import make_identity

    f32 = mybir.dt.float32
    bf16 = mybir.dt.bfloat16
    AX = mybir.AxisListType
    OP = mybir.AluOpType
    AF = mybir.ActivationFunctionType

    nc = bacc.Bacc()
    HPC = HEADS_PER_CORE

    qa_d, ka_d, qbc_d, kbc_d, va_d, out_d, diag_d = [], [], [], [], [], [], []
    for hh in range(HPC):
        se = sc[hh] * 128
        qa_d.append(nc.declare_dram_parameter(f"qa{hh}", [128, L], bf16,
                                              isOutput=False))
        ka_d.append(nc.declare_dram_parameter(f"ka{hh}", [128, se], bf16,
                                              isOutput=False))
        qbc_d.append(nc.declare_dram_parameter(f"qbc{hh}", [128, L], bf16,
                                               isOutput=False))
        kbc_d.append(nc.declare_dram_parameter(f"kbc{hh}", [128, se], bf16,
                                               isOutput=False))
        va_d.append(nc.declare_dram_parameter(f"va{hh}", [sc[hh], 128, D + 1],
                                              bf16, isOutput=False))
        out_d.append(nc.declare_dram_parameter(f"out{hh}", [L, D], f32,
                                               isOutput=True))
        diag_d.append(nc.declare_dram_parameter(f"cnt{hh}", [QB, 512],
                                                f32, isOutput=True))

    from contextlib import ExitStack
    with TileContext(nc) as tc, ExitStack() as ctx:
        consts = ctx.enter_context(tc.tile_pool(name="consts", bufs=1))
        inpool = ctx.enter_context(tc.tile_pool(name="inputs", bufs=1))
        fpool = ctx.enter_context(tc.tile_pool(name="fbuf", bufs=3))
        mpool = ctx.enter_context(tc.tile_pool(name="mbuf", bufs=2))
        small = ctx.enter_context(tc.tile_pool(name="small", bufs=2))
        gs_pool = ctx.enter_context(tc.tile_pool(name="gs", bufs=3))
        opool = ctx.enter_context(tc.tile_pool(name="outbuf", bufs=3))
        ps_f = ctx.enter_context(tc.tile_pool(name="ps_fwd", bufs=2,
                                              space="PSUM"))
        ps_t = ctx.enter_context(tc.tile_pool(name="ps_t", bufs=2,
                                              space="PSUM"))
        ps_av = ctx.enter_context(tc.tile_pool(name="ps_av", bufs=1,
                                               space="PSUM"))
        ps_x = ctx.enter_context(tc.tile_pool(name="ps_x", bufs=1,
                                              space="PSUM"))

        ident = consts.tile([128, 128], bf16)
        make_identity(nc, ident)
        ident32 = consts.tile([128, 128], f32)
        make_identity(nc, ident32)

        sel_eng = nc.gpsimd if USE_POOL_SELECT else nc.vector

        # ---- load all inputs (head 0 first so compute starts early) ----
        qa, ka, qbc, kbc, va = [], [], [], [], []
        for hh in range(HPC):
            se = sc[hh] * 128
            t = inpool.tile([128, se], bf16, tag=f"ka{hh}", name=f"ka{hh}")
            nc.sync.dma_start(t, ka_d[hh][:, :])
            ka.append(t)
            t = inpool.tile([128, L], bf16, tag=f"qa{hh}", name=f"qa{hh}")
            nc.sync.dma_start(t, qa_d[hh][:, :])
            qa.append(t)
            t = inpool.tile([128, se], bf16, tag=f"kbc{hh}", name=f"kbc{hh}")
            nc.sync.dma_start(t, kbc_d[hh][:, :])
            kbc.append(t)
            t = inpool.tile([128, L], bf16, tag=f"qbc{hh}", name=f"qbc{hh}")
            nc.sync.dma_start(t, qbc_d[hh][:, :])
            qbc.append(t)
            t = inpool.tile([128, sc[hh], D + 1], bf16, tag=f"va{hh}",
                            name=f"va{hh}")
            nc.sync.dma_start(t, va_d[hh][:, :, :].rearrange("c p d -> p c d"))
            va.append(t)

        def phase1_group(hh, g):
            """forward scores + hierarchical top-32 for tiles 4g..4g+3."""
            SC = sc[hh]
            se = SC * 128
            tcols = small.tile([128, 12], bf16, tag="tcols", name="tcols")
            dg = opool.tile([128, 4], f32, tag="dg", name="dg")
            for i in range(4):
                lt = 4 * g + i
                lhsA = qa[hh][:, lt * 128:(lt + 1) * 128]
                lhsBC = qbc[hh][:, lt * 128:(lt + 1) * 128]
                f_sb = fpool.tile([128, se], f32, tag="F", name="F")
                nblk = (se + 511) // 512
                for b in range(nblk):
                    cw = min(512, se - 512 * b)
                    cs = slice(512 * b, 512 * b + cw)
                    pf = ps_f.tile([128, 512], f32, tag="fwd", name="fwd")
                    nc.tensor.matmul(pf[:, 0:cw], lhsA, ka[hh][:, cs],
                                     start=True, stop=False)
                    nc.tensor.matmul(pf[:, 0:cw], lhsBC, kbc[hh][:, cs],
                                     start=False, stop=True)
                    nc.scalar.copy(out=f_sb[:, cs], in_=pf[:, 0:cw])
                # level 1: top-8 of each 128-chunk
                M = mpool.tile([128, SC * 8], f32, tag="M", name="M")
                for c in range(SC):
                    nc.vector.max(out=M[:, 8 * c:8 * c + 8],
                                  in_=f_sb[:, 128 * c:128 * c + 128])
                # level 2: top-32 of the candidates
                m32 = small.tile([128, 32], f32, tag="m32", name="m32")
                for r in range(4):
                    nc.vector.max(out=m32[:, 8 * r:8 * r + 8], in_=M)
                    if r < 3:
                        nc.vector.match_replace(
                            out=M, in_to_replace=m32[:, 8 * r:8 * r + 8],
                            in_values=M, imm_value=NEG)
                nc.vector.tensor_copy(dg[:, i:i + 1], m32[:, 31:32])
            # batched t-ops for the 4 tiles: m = |t|*2^-16 + 1e-37 - t
            # (= -t_minus), then bf16 triple-split of m into tcols cols
            # 0:4, 4:8, 8:12.
            aco = small.tile([128, 12], f32, tag="aco", name="aco")
            nc.scalar.activation(aco[:, 0:4], dg[:, 0:4], AF.Abs,
                                 scale=float(2.0 ** -16))
            nc.vector.scalar_tensor_tensor(
                out=aco[:, 4:8], in0=aco[:, 0:4], scalar=1e-37,
                in1=dg[:, 0:4], op0=OP.add, op1=OP.subtract)
            nc.vector.tensor_copy(tcols[:, 0:4], aco[:, 4:8])
            nc.vector.tensor_tensor(out=aco[:, 8:12], in0=aco[:, 4:8],
                                    in1=tcols[:, 0:4], op=OP.subtract)
            nc.vector.tensor_copy(tcols[:, 4:8], aco[:, 8:12])
            nc.vector.tensor_tensor(out=aco[:, 0:4], in0=aco[:, 8:12],
                                    in1=tcols[:, 4:8], op=OP.subtract)
            nc.vector.tensor_copy(tcols[:, 8:12], aco[:, 0:4])
            # transpose tcols into qa rows 65..67 of this group's columns
            pt = ps_x.tile([128, 128], bf16, tag="tposeb", name="tposeb")
            nc.tensor.transpose(pt[0:12, :], tcols, ident)
            stage = small.tile([12, 128], bf16, tag="stage12", name="stage12")
            nc.scalar.copy(out=stage, in_=pt[0:12, :])
            for j in range(3):
                nc.sync.dma_start(
                    qa[hh][65 + j:66 + j, g * 512:(g + 1) * 512].rearrange(
                        "p (t q) -> p t q", t=4),
                    stage[4 * j:4 * (j + 1), :])

        def phase2_group(hh, g):
            SC = sc[hh]
            qs = slice(g * 512, (g + 1) * 512)
            av_a = ps_av.tile([D + 1, 512], f32, tag="av_a", name="av_a")
            av_i = ps_av.tile([D + 1, 512], f32, tag="av_i", name="av_i")
            ap_t = [None] * SC
            in_t = [None] * SC

            def emit_av(c, stop):
                nc.tensor.matmul(av_a, va[hh][:, c, :], ap_t[c],
                                 start=(c == 0), stop=stop)
                nc.tensor.matmul(av_i, va[hh][:, c, :], in_t[c],
                                 start=(c == 0), stop=stop)

            for c in range(SC):
                pt = ps_t.tile([128, 512], f32, tag="psumT", name="psumT")
                nc.tensor.matmul(pt, ka[hh][:, c * 128:(c + 1) * 128],
                                 qa[hh][:, qs], start=True, stop=False)
                nc.tensor.matmul(pt, kbc[hh][:, c * 128:(c + 1) * 128],
                                 qbc[hh][:, qs], start=False, stop=True)
                # g in fp32 so that (g > 1) <=> (d' > 0) to ~1e-6; the
                # masking runs on SBUF only (GPSIMD cannot read PSUM):
                #   A'  = max(g-1, 0)   (= w-1 on selected, 0 off)
                #   M01 = (g > 1)       (selection indicator)
                # AV = V_aug^T A' + V_aug^T M01; the ones column of the
                # M01 matmul is the exact per-query selection count.
                g_sb = gs_pool.tile([128, 512], f32, tag="g", name="g")
                nc.scalar.activation(g_sb, pt, AF.Exp, scale=float(TEMP))
                ap_sb = gs_pool.tile([128, 512], bf16, tag="ap", name="ap")
                sel_eng.tensor_scalar(out=ap_sb, in0=g_sb, scalar1=1.0,
                                      scalar2=0.0, op0=OP.subtract,
                                      op1=OP.max)
                ind_sb = gs_pool.tile([128, 512], bf16, tag="ind",
                                      name="ind")
                sel_eng.tensor_scalar(out=ind_sb, in0=g_sb, scalar1=1.0,
                                      scalar2=None, op0=OP.is_gt)
                ap_t[c] = ap_sb
                in_t[c] = ind_sb
                if c >= 1:
                    emit_av(c - 1, stop=False)
            emit_av(SC - 1, stop=True)
            # copy av_i to SBUF (row D = exact count), then u = av_a + ui
            ui_sb = opool.tile([D + 1, 512], f32, tag="ui", name="ui")
            nc.scalar.copy(out=ui_sb, in_=av_i)
            nc.sync.dma_start(diag_d[hh][g], ui_sb[D:D + 1, :])
            u_sb = opool.tile([D + 1, 512], f32, tag="u", name="u")
            nc.vector.tensor_tensor(out=u_sb, in0=av_a, in1=ui_sb, op=OP.add)
            for sub in range(4):
                po = ps_x.tile([128, 128], f32, tag="tpose", name="tpose")
                nc.tensor.transpose(po[:, 0:D + 1],
                                    u_sb[:, sub * 128:(sub + 1) * 128],
                                    ident32[0:D + 1, 0:D + 1])
                recip = opool.tile([128, 1], f32, tag="recip", name="recip")
                nc.vector.reciprocal(out=recip, in_=po[:, D:D + 1])
                o_sb = opool.tile([128, D], f32, tag="osb", name="osb")
                nc.vector.tensor_scalar(
                    out=o_sb, in0=po[:, 0:D], scalar1=recip, scalar2=None,
                    op0=OP.mult)
                lq = g * 512 + sub * 128
                nc.sync.dma_start(out_d[hh][lq:lq + 128, :], o_sb)

        units = [(hh, g) for hh in range(HPC) for g in range(QB)]
        for k in range(len(units) + 1):
            if k < len(units):
                phase1_group(*units[k])
            if k >= 1:
                phase2_group(*units[k - 1])

    nc.compile()
    return nc


_NC_CACHE = {}


def _sc_of(key_lengths_i):
    return tuple(max(1, min(S, int(-(-int(key_lengths_i[n]) // 128))))
                 for n in range(N))


def _get_nc(key_lengths_i):
    key = _sc_of(key_lengths_i)
    if key not in _NC_CACHE:
        _NC_CACHE[key] = _build_bass(key)
    return _NC_CACHE[key]


def _split_hi_lo(x):
    hi = x.astype(_bf16)
    lo = (x.astype(np.float32) - hi.astype(np.float32)).astype(_bf16)
    return hi, lo


def _prep_core(core, queries, keys, values, key_lengths_i):
    """Returns (pairs, in_map) for this core.  pairs = [(n, h)] per slot."""
    sc = _sc_of(key_lengths_i)
    pairs = [(n, core) for n in range(N)]
    im = {}
    for i, (n, h) in enumerate(pairs):
        se = sc[n] * 128
        kl = int(key_lengths_i[n])
        Q = queries[n, :, h, :]             # [L, E]
        K = keys[n, :se, h, :]              # [se, E]
        V = values[n, :se, h, :]            # [se, D]
        qh, ql = _split_hi_lo(Q)
        kh, kl_ = _split_hi_lo(K)
        mask = np.where(np.arange(se) < kl, 0.0, NEG).astype(np.float32)
        qa = np.zeros((128, L), _bf16)
        ka = np.zeros((128, se), _bf16)
        qbc = np.zeros((128, L), _bf16)
        kbc = np.zeros((128, se), _bf16)
        va = np.zeros((sc[n], 128, D + 1), _bf16)
        qa[0:E, :] = qh.T
        qa[E, :] = 1.0
        # rows 65..67 stay 0 (t slots, filled on device)
        qa[E + 4:E + 4 + NLO, :] = ql.T[0:NLO]
        ka[0:E, :] = kh.T
        ka[E, :] = mask.astype(_bf16)
        ka[E + 1:E + 4, :] = 1.0
        ka[E + 4:E + 4 + NLO, :] = kl_.T[0:NLO]
        qbc[0:E, :] = qh.T
        qbc[E:2 * E, :] = ql.T
        kbc[0:E, :] = kl_.T
        kbc[E:2 * E, :] = kh.T
        va[:, :, 0:D] = V.astype(_bf16).reshape(sc[n], 128, D)
        va[:, :, D] = 1.0
        im[f"qa{i}"] = qa
        im[f"ka{i}"] = ka
        im[f"qbc{i}"] = qbc
        im[f"kbc{i}"] = kbc
        im[f"va{i}"] = va
    return pairs, im


def _host_fix_rows(out, rows_by_head, queries, keys, values, key_lengths):
    """Exact fp32 recompute (vectorized per head) of suspect rows."""
    for (n, h), rows in rows_by_head.items():
        if not rows:
            continue
        rows = np.asarray(rows, np.int64)
        kl = int(key_lengths[n])
        Qr = np.asarray(queries[n, rows, h, :], np.float32)      # [R, E]
        K = np.asarray(keys[n, :kl, h, :], np.float32)           # [kl, E]
        V = np.asarray(values[n, :kl, h, :], np.float32)         # [kl, D]
        Sc = Qr @ K.T                                            # [R, kl]
        idx = np.argpartition(-Sc, TOPK - 1, axis=1)[:, :TOPK]   # [R, 32]
        sv = np.take_along_axis(Sc, idx, axis=1)
        w = np.exp(TEMP * (sv - sv.max(axis=1, keepdims=True)))
        o = np.einsum('rk,rkd->rd', w, V[idx]) / w.sum(axis=1)[:, None]
        out[n, rows, h, :] = o


def kernel(queries, keys, values, key_lengths):
    from concourse.bass_utils import run_bass_kernel_spmd

    queries = np.asarray(queries, np.float32)
    keys = np.asarray(keys, np.float32)
    values = np.asarray(values, np.float32)
    key_lengths_i = np.asarray(key_lengths).astype(np.int64)

    in_maps = []
    head_map = []
    for core in range(N_CORES):
        pairs, im = _prep_core(core, queries, keys, values, key_lengths_i)
        head_map.append(pairs)
        in_maps.append(im)

    nc = _get_nc(key_lengths_i)
    res = run_bass_kernel_spmd(nc, in_maps, list(range(N_CORES)))

    out = np.zeros((N, L, H, D), np.float32)
    fix = {}
    for core in range(N_CORES):
        for i, (n, h) in enumerate(head_map[core]):
            out[n, :, h, :] = res.results[core][f"out{i}"].reshape(L, D)
            cnt = res.results[core][f"cnt{i}"].reshape(L)
            bad = np.nonzero(cnt != TOPK)[0]
            if len(bad):
                fix.setdefault((n, h), []).extend(int(b) for b in bad)
    if fix:
        _host_fix_rows(out, fix, queries, keys, values, key_lengths_i)
    return out


# revision 31
# speedup vs baseline: 6.5847x; 6.5847x over previous
"""Exact top-k (k=32) attention on 8 Trainium2 NeuronCores.

Head-parallel sharding: core c computes (batch 0, head c) and (batch 1,
head c).  Per-batch key-length truncation: only SC = ceil(kl/128) chunks
of 128 keys are ever touched (the rest can never enter the top-32), so
each core's two head-slots run with different (smaller) S.

Per head, per core:
  Phase 1 (selection): forward scores F[q, s] via a 2-pass bf16-split
    matmul (hi*hi + partial lo*lo in pass A; hi*lo + lo*hi in pass BC;
    ~1e-5 accurate).  Hierarchical exact top-32: per-128-chunk top-8 via
    one DVE max8 each (level 1), then top-32 of the <=128 candidates via
    4x max8 + 3x match_replace (level 2).  A row can only be mis-extracted
    if one chunk holds >= 9 of its top-32; those rows are detected via
    flag = max_c(chunk c's 8th-largest) > t and recomputed exactly on the
    host (~1% of rows).  Cut value t_minus = t - |t|*2^-23 - 1e-37,
    strictly inside (s_33, s_32]; bf16 triple-split of -t_minus is staged
    into rows 65..67 of the qa operand.
  Phase 2 (apply): transposed scores minus t_minus computed directly by
    the same augmented matmul pair (extra contraction rows carry the mask
    and -t split), giving d'[s, q] = F^T - t_minus in PSUM.  Then
      g = Exp(temp*d')            (ScalarE, bf16)
      W = (d' > 0) * g            (GpSimd scalar_tensor_tensor)
    and AV = V_aug^T W in one matmul per chunk (V_aug has a ones column
    carrying Z = sum of selected weights); output = AV / Z.
  Emission order pipelines phase 1 of unit k+1 ahead of phase 2 of unit
  k so DVE extraction overlaps PE/Scalar/Pool apply work.
"""

import numpy as np
import ml_dtypes

N, L, S, H, E, D = 2, 2048, 2048, 8, 64, 64
TOPK = 32
TEMP = 1.0 / np.sqrt(E)
HEADS_PER_CORE = 2
N_CORES = 8
LT = 16          # L tiles of 128
QB = 4           # q groups of 512
NEG = -1e30
NLO = 60         # e-rows of the lo*lo partial correction in pass A
USE_POOL_SELECT = True

_bf16 = ml_dtypes.bfloat16


def _build_bass(sc):
    """sc: tuple (SC0, SC1) chunk counts (128 keys each) per head-slot."""
    import concourse.mybir as mybir
    from concourse import bacc
    from concourse.tile import TileContext
    from concourse.masks import make_identity

    f32 = mybir.dt.float32
    bf16 = mybir.dt.bfloat16
    AX = mybir.AxisListType
    OP = mybir.AluOpType
    AF = mybir.ActivationFunctionType

    nc = bacc.Bacc()
    HPC = HEADS_PER_CORE

    qa_d, ka_d, qbc_d, kbc_d, va_d, out_d, diag_d = [], [], [], [], [], [], []
    for hh in range(HPC):
        se = sc[hh] * 128
        qa_d.append(nc.declare_dram_parameter(f"qa{hh}", [128, L], bf16,
                                              isOutput=False))
        ka_d.append(nc.declare_dram_parameter(f"ka{hh}", [128, se], bf16,
                                              isOutput=False))
        qbc_d.append(nc.declare_dram_parameter(f"qbc{hh}", [128, L], bf16,
                                               isOutput=False))
        kbc_d.append(nc.declare_dram_parameter(f"kbc{hh}", [128, se], bf16,
                                               isOutput=False))
        va_d.append(nc.declare_dram_parameter(f"va{hh}", [2, sc[hh], 128,
                                                          D + 2],
                                              bf16, isOutput=False))
        out_d.append(nc.declare_dram_parameter(f"out{hh}", [L, D], f32,
                                               isOutput=True))
        diag_d.append(nc.declare_dram_parameter(f"cnt{hh}", [QB, 128, 4],
                                                f32, isOutput=True))

    from contextlib import ExitStack
    with TileContext(nc) as tc, ExitStack() as ctx:
        consts = ctx.enter_context(tc.tile_pool(name="consts", bufs=1))
        inpool = ctx.enter_context(tc.tile_pool(name="inputs", bufs=1))
        mpool = ctx.enter_context(tc.tile_pool(name="mbuf", bufs=2))
        small = ctx.enter_context(tc.tile_pool(name="small", bufs=2))
        gs_pool = ctx.enter_context(tc.tile_pool(name="gs", bufs=3))
        opool = ctx.enter_context(tc.tile_pool(name="outbuf", bufs=3))
        ps_f = ctx.enter_context(tc.tile_pool(name="ps_fwd", bufs=3,
                                              space="PSUM"))
        ps_t = ctx.enter_context(tc.tile_pool(name="ps_t", bufs=2,
                                              space="PSUM"))
        ps_av = ctx.enter_context(tc.tile_pool(name="ps_av", bufs=1,
                                               space="PSUM"))
        ps_x = ctx.enter_context(tc.tile_pool(name="ps_x", bufs=1,
                                              space="PSUM"))

        ident = consts.tile([128, 128], bf16)
        make_identity(nc, ident)
        ident32 = consts.tile([128, 128], f32)
        make_identity(nc, ident32)
        neg1 = consts.tile([128, 1], f32)
        nc.vector.memset(neg1, -1.0)

        # ---- load all inputs (head 0 first so compute starts early) ----
        qa, ka, qbc, kbc, va = [], [], [], [], []
        for hh in range(HPC):
            se = sc[hh] * 128
            t = inpool.tile([128, se], bf16, tag=f"ka{hh}", name=f"ka{hh}")
            nc.sync.dma_start(t, ka_d[hh][:, :])
            ka.append(t)
            t = inpool.tile([128, L], bf16, tag=f"qa{hh}", name=f"qa{hh}")
            nc.sync.dma_start(t, qa_d[hh][:, :])
            qa.append(t)
            t = inpool.tile([128, se], bf16, tag=f"kbc{hh}", name=f"kbc{hh}")
            nc.sync.dma_start(t, kbc_d[hh][:, :])
            kbc.append(t)
            t = inpool.tile([128, L], bf16, tag=f"qbc{hh}", name=f"qbc{hh}")
            nc.sync.dma_start(t, qbc_d[hh][:, :])
            qbc.append(t)
            t = inpool.tile([128, 2, sc[hh], D + 2], bf16, tag=f"va{hh}",
                            name=f"va{hh}")
            nc.sync.dma_start(t, va_d[hh][:, :, :, :].rearrange(
                "a c p d -> p a c d"))
            va.append(t)

        def phase1_group(hh, g):
            """forward scores + hierarchical top-32 for tiles 4g..4g+3."""
            SC = sc[hh]
            se = SC * 128
            tcols = small.tile([128, 12], bf16, tag="tcols", name="tcols")
            dg = opool.tile([128, 4], f32, tag="dg", name="dg")
            for i in range(4):
                lt = 4 * g + i
                lhsA = qa[hh][:, lt * 128:(lt + 1) * 128]
                lhsBC = qbc[hh][:, lt * 128:(lt + 1) * 128]
                # forward scores per 512-block; level-1 top-8 per 128-chunk
                # read straight from PSUM (no SBUF staging of F needed)
                M = mpool.tile([128, SC * 8], f32, tag="M", name="M")
                nblk = (se + 511) // 512
                for b in range(nblk):
                    cw = min(512, se - 512 * b)
                    cs = slice(512 * b, 512 * b + cw)
                    pf = ps_f.tile([128, 512], f32, tag="fwd", name="fwd")
                    nc.tensor.matmul(pf[:, 0:cw], lhsA, ka[hh][:, cs],
                                     start=True, stop=False)
                    nc.tensor.matmul(pf[:, 0:cw], lhsBC, kbc[hh][:, cs],
                                     start=False, stop=True)
                    for c in range(cw // 128):
                        cc = 4 * b + c
                        nc.vector.max(out=M[:, 8 * cc:8 * cc + 8],
                                      in_=pf[:, 128 * c:128 * c + 128])
                # level 2: top-32 of the candidates
                m32 = small.tile([128, 32], f32, tag="m32", name="m32")
                for r in range(4):
                    nc.vector.max(out=m32[:, 8 * r:8 * r + 8], in_=M)
                    if r < 3:
                        nc.vector.match_replace(
                            out=M, in_to_replace=m32[:, 8 * r:8 * r + 8],
                            in_values=M, imm_value=NEG)
                nc.vector.tensor_copy(dg[:, i:i + 1], m32[:, 31:32])
            # batched t-ops for the 4 tiles: m = |t|*2^-16 + 1e-37 - t
            # (= -t_minus), then bf16 triple-split of m into tcols cols
            # 0:4, 4:8, 8:12.
            aco = small.tile([128, 12], f32, tag="aco", name="aco")
            nc.scalar.activation(aco[:, 0:4], dg[:, 0:4], AF.Abs,
                                 scale=float(2.0 ** -16))
            nc.vector.scalar_tensor_tensor(
                out=aco[:, 4:8], in0=aco[:, 0:4], scalar=1e-37,
                in1=dg[:, 0:4], op0=OP.add, op1=OP.subtract)
            nc.vector.tensor_copy(tcols[:, 0:4], aco[:, 4:8])
            nc.vector.tensor_tensor(out=aco[:, 8:12], in0=aco[:, 4:8],
                                    in1=tcols[:, 0:4], op=OP.subtract)
            nc.vector.tensor_copy(tcols[:, 4:8], aco[:, 8:12])
            nc.vector.tensor_tensor(out=aco[:, 0:4], in0=aco[:, 8:12],
                                    in1=tcols[:, 4:8], op=OP.subtract)
            nc.vector.tensor_copy(tcols[:, 8:12], aco[:, 0:4])
            # transpose tcols into qa rows 65..67 of this group's columns
            pt = ps_x.tile([128, 128], bf16, tag="tposeb", name="tposeb")
            nc.tensor.transpose(pt[0:12, :], tcols, ident)
            stage = small.tile([12, 128], bf16, tag="stage12", name="stage12")
            nc.scalar.copy(out=stage, in_=pt[0:12, :])
            for j in range(3):
                nc.sync.dma_start(
                    qa[hh][65 + j:66 + j, g * 512:(g + 1) * 512].rearrange(
                        "p (t q) -> p t q", t=4),
                    stage[4 * j:4 * (j + 1), :])

        def phase2_group(hh, g):
            SC = sc[hh]
            qs = slice(g * 512, (g + 1) * 512)
            # single accumulator: rows 0..63 AV, row 64 Z, row 65 count
            av = ps_av.tile([D + 2, 512], f32, tag="av", name="av")
            ap_t = [None] * SC
            in_t = [None] * SC

            def emit_av(c, stop):
                nc.tensor.matmul(av, va[hh][:, 0, c, :], ap_t[c],
                                 start=(c == 0), stop=False)
                nc.tensor.matmul(av, va[hh][:, 1, c, :], in_t[c],
                                 start=False, stop=stop)

            for c in range(SC):
                pt = ps_t.tile([128, 512], f32, tag="psumT", name="psumT")
                nc.tensor.matmul(pt, ka[hh][:, c * 128:(c + 1) * 128],
                                 qa[hh][:, qs], start=True, stop=False)
                nc.tensor.matmul(pt, kbc[hh][:, c * 128:(c + 1) * 128],
                                 qbc[hh][:, qs], start=False, stop=True)
                # ScalarE chain (GPSIMD is too slow, DVE is saturated):
                #   g   = exp(temp*d')   fp32 (so g>1 <=> d'>0 to ~1e-6)
                #   A'  = relu(g-1)      bf16 (= w-1 on selected, 0 off)
                #   M01 = sign(A')       bf16 in {0,1} exactly
                # AV = va_a^T A' + va_i^T M01 accumulated into one PSUM
                # tile; va col 64 = 1/1 (-> Z), col 65 = 0/1 (-> count).
                g_sb = gs_pool.tile([128, 512], f32, tag="g", name="g")
                nc.scalar.activation(g_sb, pt, AF.Exp, scale=float(TEMP))
                ap_sb = gs_pool.tile([128, 512], bf16, tag="ap", name="ap")
                nc.scalar.activation(ap_sb, g_sb, AF.Relu, bias=neg1[:, 0:1])
                ind_sb = gs_pool.tile([128, 512], bf16, tag="ind",
                                      name="ind")
                nc.scalar.activation(ind_sb, ap_sb, AF.Sign)
                ap_t[c] = ap_sb
                in_t[c] = ind_sb
                if c >= 1:
                    emit_av(c - 1, stop=False)
            emit_av(SC - 1, stop=True)
            u_sb = opool.tile([D + 2, 512], f32, tag="u", name="u")
            nc.scalar.copy(out=u_sb, in_=av)
            cnt_sb = opool.tile([128, 4], f32, tag="cnt", name="cnt")
            for sub in range(4):
                po = ps_x.tile([128, 128], f32, tag="tpose", name="tpose")
                nc.tensor.transpose(po[:, 0:D + 2],
                                    u_sb[:, sub * 128:(sub + 1) * 128],
                                    ident32[0:D + 2, 0:D + 2])
                recip = opool.tile([128, 1], f32, tag="recip", name="recip")
                nc.vector.reciprocal(out=recip, in_=po[:, D:D + 1])
                nc.vector.tensor_copy(cnt_sb[:, sub:sub + 1],
                                      po[:, D + 1:D + 2])
                o_sb = opool.tile([128, D], f32, tag="osb", name="osb")
                nc.vector.tensor_scalar(
                    out=o_sb, in0=po[:, 0:D], scalar1=recip, scalar2=None,
                    op0=OP.mult)
                lq = g * 512 + sub * 128
                nc.sync.dma_start(out_d[hh][lq:lq + 128, :], o_sb)
            nc.sync.dma_start(diag_d[hh][g], cnt_sb)

        units = [(hh, g) for hh in range(HPC) for g in range(QB)]
        for k in range(len(units) + 1):
            if k < len(units):
                phase1_group(*units[k])
            if k >= 1:
                phase2_group(*units[k - 1])

    nc.compile()
    return nc


_NC_CACHE = {}


def _sc_of(key_lengths_i):
    return tuple(max(1, min(S, int(-(-int(key_lengths_i[n]) // 128))))
                 for n in range(N))


def _get_nc(key_lengths_i):
    key = _sc_of(key_lengths_i)
    if key not in _NC_CACHE:
        _NC_CACHE[key] = _build_bass(key)
    return _NC_CACHE[key]


def _split_hi_lo(x):
    hi = x.astype(_bf16)
    lo = (x.astype(np.float32) - hi.astype(np.float32)).astype(_bf16)
    return hi, lo


def _prep_core(core, queries, keys, values, key_lengths_i):
    """Returns (pairs, in_map) for this core.  pairs = [(n, h)] per slot."""
    sc = _sc_of(key_lengths_i)
    pairs = [(n, core) for n in range(N)]
    im = {}
    for i, (n, h) in enumerate(pairs):
        se = sc[n] * 128
        kl = int(key_lengths_i[n])
        Q = queries[n, :, h, :]             # [L, E]
        K = keys[n, :se, h, :]              # [se, E]
        V = values[n, :se, h, :]            # [se, D]
        qh, ql = _split_hi_lo(Q)
        kh, kl_ = _split_hi_lo(K)
        mask = np.where(np.arange(se) < kl, 0.0, NEG).astype(np.float32)
        qa = np.zeros((128, L), _bf16)
        ka = np.zeros((128, se), _bf16)
        qbc = np.zeros((128, L), _bf16)
        kbc = np.zeros((128, se), _bf16)
        va = np.zeros((2, sc[n], 128, D + 2), _bf16)
        qa[0:E, :] = qh.T
        qa[E, :] = 1.0
        # rows 65..67 stay 0 (t slots, filled on device)
        qa[E + 4:E + 4 + NLO, :] = ql.T[0:NLO]
        ka[0:E, :] = kh.T
        ka[E, :] = mask.astype(_bf16)
        ka[E + 1:E + 4, :] = 1.0
        ka[E + 4:E + 4 + NLO, :] = kl_.T[0:NLO]
        qbc[0:E, :] = qh.T
        qbc[E:2 * E, :] = ql.T
        kbc[0:E, :] = kl_.T
        kbc[E:2 * E, :] = kh.T
        va[:, :, :, 0:D] = V.astype(_bf16).reshape(sc[n], 128, D)[None]
        va[:, :, :, D] = 1.0      # -> row 64: Z = sum of weights
        va[1, :, :, D + 1] = 1.0  # -> row 65: exact selection count
        im[f"qa{i}"] = qa
        im[f"ka{i}"] = ka
        im[f"qbc{i}"] = qbc
        im[f"kbc{i}"] = kbc
        im[f"va{i}"] = va
    return pairs, im


def _host_fix_rows(out, rows_by_head, queries, keys, values, key_lengths):
    """Exact fp32 recompute (vectorized per head) of suspect rows."""
    for (n, h), rows in rows_by_head.items():
        if not rows:
            continue
        rows = np.asarray(rows, np.int64)
        kl = int(key_lengths[n])
        Qr = np.asarray(queries[n, rows, h, :], np.float32)      # [R, E]
        K = np.asarray(keys[n, :kl, h, :], np.float32)           # [kl, E]
        V = np.asarray(values[n, :kl, h, :], np.float32)         # [kl, D]
        Sc = Qr @ K.T                                            # [R, kl]
        idx = np.argpartition(-Sc, TOPK - 1, axis=1)[:, :TOPK]   # [R, 32]
        sv = np.take_along_axis(Sc, idx, axis=1)
        w = np.exp(TEMP * (sv - sv.max(axis=1, keepdims=True)))
        o = np.einsum('rk,rkd->rd', w, V[idx]) / w.sum(axis=1)[:, None]
        out[n, rows, h, :] = o


def kernel(queries, keys, values, key_lengths):
    from concourse.bass_utils import run_bass_kernel_spmd

    queries = np.asarray(queries, np.float32)
    keys = np.asarray(keys, np.float32)
    values = np.asarray(values, np.float32)
    key_lengths_i = np.asarray(key_lengths).astype(np.int64)

    in_maps = []
    head_map = []
    for core in range(N_CORES):
        pairs, im = _prep_core(core, queries, keys, values, key_lengths_i)
        head_map.append(pairs)
        in_maps.append(im)

    nc = _get_nc(key_lengths_i)
    res = run_bass_kernel_spmd(nc, in_maps, list(range(N_CORES)))

    out = np.zeros((N, L, H, D), np.float32)
    fix = {}
    for core in range(N_CORES):
        for i, (n, h) in enumerate(head_map[core]):
            out[n, :, h, :] = res.results[core][f"out{i}"].reshape(L, D)
            cnt = res.results[core][f"cnt{i}"].reshape(QB, 128, 4)
            cnt = cnt.transpose(0, 2, 1).reshape(L)
            bad = np.nonzero(cnt != TOPK)[0]
            if len(bad):
                fix.setdefault((n, h), []).extend(int(b) for b in bad)
    if fix:
        _host_fix_rows(out, fix, queries, keys, values, key_lengths_i)
    return out
